# revision 18
# baseline (speedup 1.0000x reference)
"""Trainium2 Bass kernel for nn_Jurassic3Mamba (Mamba-1 forward), 8-core SPMD.

v2: chunk-pipelined (512-token chunks), tensor-parallel over d_inner.
- All scan-phase elementwise ops in bf16 on DVE (no gpsimd -> no SBUF-port
  contention), dA on the scalar (ACT) engine.
- y = sum_n h_n*C_n accumulated in PSUM via identity-weight matmuls on the
  tensor engine (frees the vector engine of 15 adds per tile).
- AllReduce of x_dbl in bf16, one collective per 512-token chunk, launched
  ~75% through the previous chunk's scan so its latency is hidden.
- Native Silu / Softplus activations (one ACT op instead of sigmoid+mul /
  exp+ln pairs).
"""
import sys
if "/opt/trn_rl_repo" not in sys.path:
    sys.path.insert(0, "/opt/trn_rl_repo")


from contextlib import ExitStack

import concourse.bass as bass
import concourse.mybir as mybir
import concourse.tile as tile

FP32 = mybir.dt.float32
BF16 = mybir.dt.bfloat16
ALU = mybir.AluOpType
ACTF = mybir.ActivationFunctionType


class Cfg:
    def __init__(self, DM=2048, DC=512, N=16, R=128, TOK=2048, L=1024,
                 n_cores=8, scan_fd=256):
        self.DM = DM          # d_model
        self.DC = DC          # d_inner per core
        self.N = N            # d_state
        self.R = R            # dt_rank
        self.TOK = TOK        # B * L tokens
        self.L = L            # seq len per batch
        self.CH = 512         # chunk tokens
        self.n_cores = n_cores
        self.scan_fd = scan_fd
        assert DM % 128 == 0 and DC % 128 == 0 and R == 128
        self.KT = DM // 128   # k-tiles for in_proj contraction
        self.DT = DC // 128   # d-tiles per core
        self.NCH = TOK // self.CH  # chunks


def declare_io(nc, cfg):
    DM, DC, N, R, TOK = cfg.DM, cfg.DC, cfg.N, cfg.R, cfg.TOK
    io = {}
    io["hsT"] = nc.dram_tensor("hsT", [DM, TOK], BF16, kind="ExternalInput")
    io["wxT"] = nc.dram_tensor("wxT", [DM, DC], BF16, kind="ExternalInput")
    io["wzT"] = nc.dram_tensor("wzT", [DM, DC], BF16, kind="ExternalInput")
    io["xpT"] = nc.dram_tensor("xpT", [DC, R + 2 * N], BF16, kind="ExternalInput")
    io["dtpT"] = nc.dram_tensor("dtpT", [R, DC], BF16, kind="ExternalInput")
    io["woT"] = nc.dram_tensor("woT", [DC, DM], BF16, kind="ExternalInput")
    io["convw"] = nc.dram_tensor("convw", [DC, 4], FP32, kind="ExternalInput")
    io["convb"] = nc.dram_tensor("convb", [DC, 1], FP32, kind="ExternalInput")
    io["Amat"] = nc.dram_tensor("Amat", [DC, N], FP32, kind="ExternalInput")
    io["Dvec"] = nc.dram_tensor("Dvec", [DC, 1], FP32, kind="ExternalInput")
    io["dtb"] = nc.dram_tensor("dtb", [DC, 1], FP32, kind="ExternalInput")
    io["ident"] = nc.dram_tensor("ident", [128, 128], BF16, kind="ExternalInput")
    io["outp"] = nc.dram_tensor("outp", [TOK, DM], FP32, kind="ExternalOutput")
    return io


def build(tc: tile.TileContext, io, cfg: Cfg):
    nc = tc.nc
    ctx = ExitStack()
    DM, DC, N, R, TOK, L, CH = cfg.DM, cfg.DC, cfg.N, cfg.R, cfg.TOK, cfg.L, cfg.CH
    KT, DT, NCH = cfg.KT, cfg.DT, cfg.NCH
    HF = cfg.scan_fd  # scan segment length

    persist = ctx.enter_context(tc.tile_pool(name="persist", bufs=1))
    dram = ctx.enter_context(tc.tile_pool(name="dram", bufs=1, space="DRAM"))

    # ---- persistent weights ----
    xp_sb = persist.tile([128, DT, R + 2 * N], BF16, tag="xp")
    nc.sync.dma_start(xp_sb[:], io["xpT"].ap().rearrange("(t p) c -> p t c", p=128))
    dtp_sb = persist.tile([128, DC], BF16, tag="dtp")
    nc.sync.dma_start(dtp_sb[:], io["dtpT"].ap())
    wo_sb = persist.tile([128, DT, DM], BF16, tag="wo")
    nc.sync.dma_start(wo_sb[:], io["woT"].ap().rearrange("(t p) m -> p t m", p=128))
    wx_sb = persist.tile([128, KT, DC], BF16, tag="wx")
    nc.sync.dma_start(wx_sb[:], io["wxT"].ap().rearrange("(t p) c -> p t c", p=128))
    wz_sb = persist.tile([128, KT, DC], BF16, tag="wz")
    nc.sync.dma_start(wz_sb[:], io["wzT"].ap().rearrange("(t p) c -> p t c", p=128))
    convw_sb = persist.tile([128, DT, 4], FP32, tag="convw")
    nc.sync.dma_start(convw_sb[:], io["convw"].ap().rearrange("(t p) k -> p t k", p=128))
    convb_sb = persist.tile([128, DT, 1], FP32, tag="convb")
    nc.sync.dma_start(convb_sb[:], io["convb"].ap().rearrange("(t p) k -> p t k", p=128))
    A_sb = persist.tile([128, DT, N], FP32, tag="A")
    nc.sync.dma_start(A_sb[:], io["Amat"].ap().rearrange("(t p) n -> p t n", p=128))
    Dv_sb = persist.tile([128, DT, 1], FP32, tag="Dv")
    nc.sync.dma_start(Dv_sb[:], io["Dvec"].ap().rearrange("(t p) k -> p t k", p=128))
    dtb_sb = persist.tile([128, DT, 1], FP32, tag="dtb")
    nc.sync.dma_start(dtb_sb[:], io["dtb"].ap().rearrange("(t p) k -> p t k", p=128))
    id_sb = persist.tile([128, 128], BF16, tag="ident")
    nc.sync.dma_start(id_sb[:], io["ident"].ap())

    # persistent activations [128, TOK] bf16 per d-tile
    xpre = [persist.tile([128, TOK], BF16, tag=f"xpre{i}", name=f"xpre{i}") for i in range(DT)]
    xact = [persist.tile([128, TOK], BF16, tag=f"xact{i}", name=f"xact{i}") for i in range(DT)]
    sz = [persist.tile([128, TOK], BF16, tag=f"sz{i}", name=f"sz{i}") for i in range(DT)]
    dt_sb = [persist.tile([128, TOK], BF16, tag=f"dt{i}", name=f"dt{i}") for i in range(DT)]
    htail = persist.tile([128, DT * N], BF16, tag="htail")

    # per-chunk DRAM bounce buffers for the collective
    xdb_part = [dram.tile([R + 2 * N, CH], BF16, name=f"xdbp{c}") for c in range(NCH)]
    xdb_red = [dram.tile([R + 2 * N, CH], BF16, addr_space="Shared", name=f"xdbr{c}")
               for c in range(NCH)]

    hsT = io["hsT"].ap().rearrange("(t p) tok -> t p tok", p=128)  # [KT,128,TOK]
    outp = io["outp"].ap()

    # ---- working pools (whole-kernel scope) ----
    hs_pool = ctx.enter_context(tc.tile_pool(name="hs", bufs=6))
    bc_pool = ctx.enter_context(tc.tile_pool(name="bc", bufs=1))
    dtin_pool = ctx.enter_context(tc.tile_pool(name="dtin", bufs=2))
    dA_pool = ctx.enter_context(tc.tile_pool(name="dA", bufs=3))
    dbx_pool = ctx.enter_context(tc.tile_pool(name="dbx", bufs=2))
    h_pool = ctx.enter_context(tc.tile_pool(name="h", bufs=3))
    hc_pool = ctx.enter_context(tc.tile_pool(name="hc", bufs=12))
    yg_pool = ctx.enter_context(tc.tile_pool(name="ygp", bufs=2))
    misc_pool = ctx.enter_context(tc.tile_pool(name="misc", bufs=2))
    psA = ctx.enter_context(tc.tile_pool(name="psA", bufs=2, space="PSUM"))
    psS = ctx.enter_context(tc.tile_pool(name="psS", bufs=2, space="PSUM"))
    psX = ctx.enter_context(tc.tile_pool(name="psX", bufs=1, space="PSUM"))
    psO = ctx.enter_context(tc.tile_pool(name="psO", bufs=2, space="PSUM"))

    yacc_live = {}  # i -> psum tile for current chunk
    yg_live = {}    # i -> per-chunk gated-output tile [128, CH]
    dtx_live = {}   # i -> per-chunk dt*x tile [128, CH]

    def in_proj(c, i):
        """x/z projections for chunk c, d-tile i -> xpre[i], sz[i]."""
        csl = slice(c * CH, (c + 1) * CH)
        dsl = slice(i * 128, (i + 1) * 128)
        psx = psA.tile([128, CH], FP32, tag="inp", name=f"psx{c}_{i}")
        psz = psA.tile([128, CH], FP32, tag="inp", name=f"psz{c}_{i}")
        for ki in range(KT):
            hst = hs_pool.tile([128, CH], BF16, tag="hs")
            nc.sync.dma_start(hst[:], hsT[ki, :, csl])
            st = (ki == 0)
            sp = (ki == KT - 1)
            nc.tensor.matmul(psx[:], wx_sb[:, ki, dsl], hst[:], start=st, stop=sp)
            nc.tensor.matmul(psz[:], wz_sb[:, ki, dsl], hst[:], start=st, stop=sp)
        nc.scalar.copy(xpre[i][:, csl], psx[:])
        nc.scalar.activation(sz[i][:, csl], psz[:], ACTF.Silu)

    def conv(c, i):
        """causal depthwise conv over chunk c for d-tile i -> xact[i]."""
        bs = c * CH
        obs = bs % L  # offset within the batch
        acc = misc_pool.tile([128, CH], BF16, tag="cacc")
        nc.vector.tensor_scalar(acc[:], xpre[i][:, bs:bs + CH],
                                convw_sb[:, i, 3:4], convb_sb[:, i, :],
                                op0=ALU.mult, op1=ALU.add)
        for sh in (1, 2, 3):
            w = convw_sb[:, i, 3 - sh:4 - sh]
            if obs >= sh:
                nc.vector.scalar_tensor_tensor(
                    acc[:], xpre[i][:, bs - sh:bs + CH - sh], w, acc[:],
                    op0=ALU.mult, op1=ALU.add)
            else:
                nc.vector.scalar_tensor_tensor(
                    acc[:, sh:], xpre[i][:, bs:bs + CH - sh], w, acc[:, sh:],
                    op0=ALU.mult, op1=ALU.add)
        nc.scalar.activation(xact[i][:, bs:bs + CH], acc[:], ACTF.Silu)

    def x_proj_ar(c):
        """x_proj partials + chunked AllReduce for chunk c."""
        csl = slice(c * CH, (c + 1) * CH)
        ps0 = psX.tile([128, CH], FP32, tag="xpb")
        ps1 = psX.tile([2 * N, CH], FP32, tag="xps")
        for i in range(DT):
            nc.tensor.matmul(ps0[:], xp_sb[:, i, :R], xact[i][:, csl],
                             start=(i == 0), stop=(i == DT - 1))
            nc.tensor.matmul(ps1[:], xp_sb[:, i, R:], xact[i][:, csl],
                             start=(i == 0), stop=(i == DT - 1))
        st0 = misc_pool.tile([128, CH], BF16, tag="xst0")
        nc.scalar.copy(st0[:], ps0[:])
        st1 = misc_pool.tile([2 * N, CH], BF16, tag="xst1")
        nc.scalar.copy(st1[:], ps1[:])
        nc.sync.dma_start(xdb_part[c][:R, :], st0[:])
        nc.sync.dma_start(xdb_part[c][R:, :], st1[:])
        nc.gpsimd.collective_compute(
            "AllReduce", ALU.add,
            replica_groups=[list(range(cfg.n_cores))],
            ins=[xdb_part[c].opt()], outs=[xdb_red[c].opt()])

    def dt_proj(c):
        """dt_proj + softplus + dtx for chunk c."""
        csl = slice(c * CH, (c + 1) * CH)
        dtin = dtin_pool.tile([128, CH], BF16, tag="dtin")
        nc.sync.dma_start(dtin[:], xdb_red[c][:R, :])
        for i in range(DT):
            dsl = slice(i * 128, (i + 1) * 128)
            psd = psX.tile([128, CH], FP32, tag="xpb", name=f"psdt{c}_{i}")
            nc.tensor.matmul(psd[:], dtp_sb[:, dsl], dtin[:], start=True, stop=True)
            # softplus(x) = ln(1 + exp(x)); Exp and Ln share one act table
            et = misc_pool.tile([128, CH], FP32, tag="spexp")
            nc.scalar.activation(et[:], psd[:], ACTF.Exp, bias=dtb_sb[:, i, :])
            nc.scalar.activation(dt_sb[i][:, csl], et[:], ACTF.Ln, bias=1.0)
        for i in range(DT):
            dtxt = yg_pool.tile([128, CH], BF16, tag=f"dtx{i}", name=f"dtx{c}_{i}")
            dtx_live[i] = dtxt
            nc.vector.tensor_mul(dtxt[:], dt_sb[i][:, csl], xact[i][:, csl])

    def bcast(c):
        """broadcast B and C rows for chunk c across partitions."""
        bcb = bc_pool.tile([128, N, CH], BF16, tag="bcb", name=f"bcb{c}")
        bcc = bc_pool.tile([128, N, CH], BF16, tag="bcc", name=f"bcc{c}")
        for n in range(N):
            nc.sync.dma_start(bcb[:, n, :],
                              xdb_red[c][R + n:R + n + 1, :].to_broadcast((128, CH)))
            nc.sync.dma_start(bcc[:, n, :],
                              xdb_red[c][R + N + n:R + N + n + 1, :].to_broadcast((128, CH)))
        return bcb, bcc

    def scan_block(c, i, bcb, bcc):
        """16-state scan for chunk c, d-tile i; y accumulated into PSUM."""
        csl = slice(c * CH, (c + 1) * CH)
        yacc = psS.tile([128, CH], FP32, tag="yacc", name=f"yacc{c}_{i}")
        yacc_live[i] = yacc
        for n in range(N):
            dA = dA_pool.tile([128, CH], BF16, tag="dA")
            nc.scalar.activation(dA[:], dt_sb[i][:, csl], ACTF.Exp,
                                 scale=A_sb[:, i, n:n + 1])
            dBx = dbx_pool.tile([128, CH], BF16, tag="dBx")
            nc.vector.tensor_mul(dBx[:], dtx_live[i][:], bcb[:, n, :])
            h = h_pool.tile([128, CH], BF16, tag="h")
            hcol = i * N + n
            for s0 in range(0, CH, HF):
                seg = slice(s0, s0 + HF)
                if s0 == 0:
                    init = 0.0 if c % 2 == 0 else htail[:, hcol:hcol + 1]
                else:
                    init = h[:, s0 - 1:s0]
                nc.vector.tensor_tensor_scan(h[:, seg], dA[:, seg], dBx[:, seg],
                                             init, op0=ALU.mult, op1=ALU.add)
            if c % 2 == 0:
                nc.vector.tensor_copy(htail[:, hcol:hcol + 1], h[:, CH - 1:CH])
            hC = hc_pool.tile([128, CH], BF16, tag="hC")
            nc.vector.tensor_mul(hC[:], h[:], bcc[:, n, :])
            nc.tensor.matmul(yacc[:], id_sb[:], hC[:],
                             start=(n == 0), stop=(n == N - 1))

    def gating(c, i):
        """yg = (yacc + xact*D) * silu(z) for chunk c, d-tile i."""
        csl = slice(c * CH, (c + 1) * CH)
        tmp = misc_pool.tile([128, CH], BF16, tag="gtmp")
        nc.vector.scalar_tensor_tensor(tmp[:], xact[i][:, csl], Dv_sb[:, i, :],
                                       yacc_live[i][:], op0=ALU.mult, op1=ALU.add)
        ygt = yg_pool.tile([128, CH], BF16, tag=f"yg{i}", name=f"yg{c}_{i}")
        yg_live[i] = ygt
        nc.vector.tensor_mul(ygt[:], tmp[:], sz[i][:, csl])

    def out_proj(c):
        """out_proj for chunk c's tokens."""
        for tt in range(CH // 128):
            tok0 = c * CH + tt * 128
            tsl = slice(tt * 128, (tt + 1) * 128)
            for mc in range(DM // 512):
                msl = slice(mc * 512, (mc + 1) * 512)
                po = psO.tile([128, 512], FP32, tag="po")
                for i in range(DT):
                    nc.tensor.matmul(po[:], yg_live[i][:, tsl],
                                     wo_sb[:, i, msl],
                                     start=(i == 0), stop=(i == DT - 1))
                ost = misc_pool.tile([128, 512], FP32, tag="ost")
                nc.scalar.copy(ost[:], po[:])
                nc.sync.dma_start(outp[tok0:tok0 + 128, msl], ost[:])

    # ================= emission =================
    # prologue: chunk 0 front-end
    for i in range(DT):
        in_proj(0, i)
        conv(0, i)
    x_proj_ar(0)
    dt_proj(0)
    bc = bcast(0)

    for c in range(NCH):
        nxt = c + 1
        scan_block(c, 0, *bc)
        if nxt < NCH:
            in_proj(nxt, 0)
            conv(nxt, 0)
            in_proj(nxt, 1)
            conv(nxt, 1)
        gating(c, 0)
        scan_block(c, 1, *bc)
        gating(c, 1)
        scan_block(c, 2, *bc)
        if nxt < NCH:
            in_proj(nxt, 2)
            conv(nxt, 2)
            in_proj(nxt, 3)
            conv(nxt, 3)
            x_proj_ar(nxt)
        gating(c, 2)
        scan_block(c, 3, *bc)
        if nxt < NCH:
            dt_proj(nxt)
            bc = bcast(nxt)
        gating(c, 3)
        out_proj(c)

    ctx.close()


# ===================== driver =====================
import numpy as np
import ml_dtypes

_N_CORES = 8
_B, _L, _DM = 2, 1024, 2048
_DI = 2 * _DM
_DC = _DI // _N_CORES
_N_STATE = 16
_R = _DM // 16

_compiled = None


def _get_compiled():
    global _compiled
    if _compiled is not None:
        return _compiled
    import concourse.bacc as bacc
    import concourse.tile as tile_mod
    cfg = Cfg(DM=_DM, DC=_DC, N=_N_STATE, R=_R, TOK=_B * _L, L=_L,
              n_cores=_N_CORES)
    nc = bacc.Bacc("TRN2", target_bir_lowering=False, debug=False,
                   num_devices=_N_CORES)
    io = declare_io(nc, cfg)
    with tile_mod.TileContext(nc) as tc:
        build(tc, io, cfg)
    nc.compile()
    _compiled = (nc, cfg)
    return _compiled


def _prep_in_maps(hidden_states, in_proj_w, conv_w, conv_b, x_proj_w,
                  dt_proj_w, dt_proj_b, A_log, D, out_proj_w):
    f32 = np.float32
    bf16 = ml_dtypes.bfloat16
    hs = np.ascontiguousarray(np.asarray(hidden_states, f32).reshape(_B * _L, _DM).T)
    in_proj_w = np.asarray(in_proj_w, f32)
    A = -np.exp(np.asarray(A_log, f32))
    x_proj_w = np.asarray(x_proj_w, f32)
    dt_proj_w = np.asarray(dt_proj_w, f32)
    out_proj_w = np.asarray(out_proj_w, f32)
    conv_w = np.asarray(conv_w, f32)
    conv_b = np.asarray(conv_b, f32)
    dt_proj_b = np.asarray(dt_proj_b, f32)
    D = np.asarray(D, f32)
    ident = np.eye(128, dtype=bf16)
    in_maps = []
    for c in range(_N_CORES):
        sl = slice(c * _DC, (c + 1) * _DC)
        in_maps.append({
            "hsT": hs.astype(bf16),
            "wxT": np.ascontiguousarray(in_proj_w[:_DI][sl].T).astype(bf16),
            "wzT": np.ascontiguousarray(in_proj_w[_DI:][sl].T).astype(bf16),
            "xpT": np.ascontiguousarray(x_proj_w[:, sl].T).astype(bf16),
            "dtpT": np.ascontiguousarray(dt_proj_w[sl].T).astype(bf16),
            "woT": np.ascontiguousarray(out_proj_w[:, sl].T).astype(bf16),
            "convw": np.ascontiguousarray(conv_w[sl]),
            "convb": np.ascontiguousarray(conv_b[sl][:, None]),
            "Amat": np.ascontiguousarray(A[sl]),
            "Dvec": np.ascontiguousarray(D[sl][:, None]),
            "dtb": np.ascontiguousarray(dt_proj_b[sl][:, None]),
            "ident": ident,
        })
    return in_maps


def kernel_run(trace=False, **inputs):
    from concourse import bass_utils
    nc, cfg = _get_compiled()
    in_maps = _prep_in_maps(**inputs)
    res = bass_utils.run_bass_kernel_spmd(
        nc, in_maps, core_ids=list(range(_N_CORES)), trace=trace)
    out = np.zeros((_B * _L, _DM), np.float64)
    for r in res.results:
        out += r["outp"].astype(np.float64)
    full = out.astype(np.float32).reshape(_B, _L, _DM)
    return full, res


def kernel(**inputs):
    full, _ = kernel_run(trace=False, **inputs)
    return full


# revision 19
# speedup vs baseline: 1.0876x; 1.0876x over previous
"""Trainium2 Bass kernel for nn_Jurassic3Mamba (Mamba-1 forward), 8-core SPMD.

v2: chunk-pipelined (512-token chunks), tensor-parallel over d_inner.
- All scan-phase elementwise ops in bf16 on DVE (no gpsimd -> no SBUF-port
  contention), dA on the scalar (ACT) engine.
- y = sum_n h_n*C_n accumulated in PSUM via identity-weight matmuls on the
  tensor engine (frees the vector engine of 15 adds per tile).
- AllReduce of x_dbl in bf16, one collective per 512-token chunk, launched
  ~75% through the previous chunk's scan so its latency is hidden.
- Native Silu / Softplus activations (one ACT op instead of sigmoid+mul /
  exp+ln pairs).
"""
import sys
if "/opt/trn_rl_repo" not in sys.path:
    sys.path.insert(0, "/opt/trn_rl_repo")


from contextlib import ExitStack

import concourse.bass as bass
import concourse.mybir as mybir
import concourse.tile as tile

FP32 = mybir.dt.float32
BF16 = mybir.dt.bfloat16
ALU = mybir.AluOpType
ACTF = mybir.ActivationFunctionType


class Cfg:
    def __init__(self, DM=2048, DC=512, N=16, R=128, TOK=2048, L=1024,
                 n_cores=8, scan_fd=256):
        self.DM = DM          # d_model
        self.DC = DC          # d_inner per core
        self.N = N            # d_state
        self.R = R            # dt_rank
        self.TOK = TOK        # B * L tokens
        self.L = L            # seq len per batch
        self.CH = 512         # chunk tokens
        self.n_cores = n_cores
        self.scan_fd = scan_fd
        assert DM % 128 == 0 and DC % 128 == 0 and R == 128
        self.KT = DM // 128   # k-tiles for in_proj contraction
        self.DT = DC // 128   # d-tiles per core
        self.NCH = TOK // self.CH  # chunks


def declare_io(nc, cfg):
    DM, DC, N, R, TOK = cfg.DM, cfg.DC, cfg.N, cfg.R, cfg.TOK
    io = {}
    io["hsT"] = nc.dram_tensor("hsT", [DM, TOK], BF16, kind="ExternalInput")
    io["wxT"] = nc.dram_tensor("wxT", [DM, DC], BF16, kind="ExternalInput")
    io["wzT"] = nc.dram_tensor("wzT", [DM, DC], BF16, kind="ExternalInput")
    io["xpT"] = nc.dram_tensor("xpT", [DC, R + 2 * N], BF16, kind="ExternalInput")
    io["dtpT"] = nc.dram_tensor("dtpT", [R, DC], BF16, kind="ExternalInput")
    io["woT"] = nc.dram_tensor("woT", [DC, DM], BF16, kind="ExternalInput")
    io["convw"] = nc.dram_tensor("convw", [DC, 4], FP32, kind="ExternalInput")
    io["convb"] = nc.dram_tensor("convb", [DC, 1], FP32, kind="ExternalInput")
    io["Amat"] = nc.dram_tensor("Amat", [DC, N], FP32, kind="ExternalInput")
    io["Dvec"] = nc.dram_tensor("Dvec", [DC, 1], FP32, kind="ExternalInput")
    io["dtb"] = nc.dram_tensor("dtb", [DC, 1], FP32, kind="ExternalInput")
    io["ident"] = nc.dram_tensor("ident", [128, 128], BF16, kind="ExternalInput")
    io["outp"] = nc.dram_tensor("outp", [TOK, DM], FP32, kind="ExternalOutput")
    return io


def build(tc: tile.TileContext, io, cfg: Cfg):
    nc = tc.nc
    ctx = ExitStack()
    DM, DC, N, R, TOK, L, CH = cfg.DM, cfg.DC, cfg.N, cfg.R, cfg.TOK, cfg.L, cfg.CH
    KT, DT, NCH = cfg.KT, cfg.DT, cfg.NCH
    HF = cfg.scan_fd  # scan segment length

    persist = ctx.enter_context(tc.tile_pool(name="persist", bufs=1))
    dram = ctx.enter_context(tc.tile_pool(name="dram", bufs=1, space="DRAM"))

    # ---- persistent weights ----
    xp_sb = persist.tile([128, DT, R + 2 * N], BF16, tag="xp")
    nc.sync.dma_start(xp_sb[:], io["xpT"].ap().rearrange("(t p) c -> p t c", p=128))
    dtp_sb = persist.tile([128, DC], BF16, tag="dtp")
    nc.sync.dma_start(dtp_sb[:], io["dtpT"].ap())
    wo_sb = persist.tile([128, DT, DM], BF16, tag="wo")
    nc.sync.dma_start(wo_sb[:], io["woT"].ap().rearrange("(t p) m -> p t m", p=128))
    wx_sb = persist.tile([128, KT, DC], BF16, tag="wx")
    nc.sync.dma_start(wx_sb[:], io["wxT"].ap().rearrange("(t p) c -> p t c", p=128))
    wz_sb = persist.tile([128, KT, DC], BF16, tag="wz")
    nc.sync.dma_start(wz_sb[:], io["wzT"].ap().rearrange("(t p) c -> p t c", p=128))
    convw_sb = persist.tile([128, DT, 4], FP32, tag="convw")
    nc.sync.dma_start(convw_sb[:], io["convw"].ap().rearrange("(t p) k -> p t k", p=128))
    convb_sb = persist.tile([128, DT, 1], FP32, tag="convb")
    nc.sync.dma_start(convb_sb[:], io["convb"].ap().rearrange("(t p) k -> p t k", p=128))
    A_sb = persist.tile([128, DT, N], FP32, tag="A")
    nc.sync.dma_start(A_sb[:], io["Amat"].ap().rearrange("(t p) n -> p t n", p=128))
    Dv_sb = persist.tile([128, DT, 1], FP32, tag="Dv")
    nc.sync.dma_start(Dv_sb[:], io["Dvec"].ap().rearrange("(t p) k -> p t k", p=128))
    dtb_sb = persist.tile([128, DT, 1], FP32, tag="dtb")
    nc.sync.dma_start(dtb_sb[:], io["dtb"].ap().rearrange("(t p) k -> p t k", p=128))
    id_sb = persist.tile([128, 128], BF16, tag="ident")
    nc.sync.dma_start(id_sb[:], io["ident"].ap())

    # persistent activations [128, TOK] bf16 per d-tile
    xpre = [persist.tile([128, TOK], BF16, tag=f"xpre{i}", name=f"xpre{i}") for i in range(DT)]
    xact = [persist.tile([128, TOK], BF16, tag=f"xact{i}", name=f"xact{i}") for i in range(DT)]
    sz = [persist.tile([128, TOK], BF16, tag=f"sz{i}", name=f"sz{i}") for i in range(DT)]
    dt_sb = [persist.tile([128, TOK], BF16, tag=f"dt{i}", name=f"dt{i}") for i in range(DT)]
    htail = persist.tile([128, DT * N], BF16, tag="htail")

    # per-chunk DRAM bounce buffers for the collective
    xdb_part = [dram.tile([R + 2 * N, CH], BF16, name=f"xdbp{c}") for c in range(NCH)]
    xdb_red = [dram.tile([R + 2 * N, CH], BF16, addr_space="Shared", name=f"xdbr{c}")
               for c in range(NCH)]

    hsT = io["hsT"].ap().rearrange("(t p) tok -> t p tok", p=128)  # [KT,128,TOK]
    outp = io["outp"].ap()

    # ---- working pools (whole-kernel scope) ----
    hs_pool = ctx.enter_context(tc.tile_pool(name="hs", bufs=6))
    bc_pool = ctx.enter_context(tc.tile_pool(name="bc", bufs=1))
    dtin_pool = ctx.enter_context(tc.tile_pool(name="dtin", bufs=2))
    dA_pool = ctx.enter_context(tc.tile_pool(name="dA", bufs=3))
    dbx_pool = ctx.enter_context(tc.tile_pool(name="dbx", bufs=2))
    h_pool = ctx.enter_context(tc.tile_pool(name="h", bufs=3))
    hc_pool = ctx.enter_context(tc.tile_pool(name="hc", bufs=12))
    yg_pool = ctx.enter_context(tc.tile_pool(name="ygp", bufs=2))
    misc_pool = ctx.enter_context(tc.tile_pool(name="misc", bufs=2))
    psA = ctx.enter_context(tc.tile_pool(name="psA", bufs=2, space="PSUM"))
    psS = ctx.enter_context(tc.tile_pool(name="psS", bufs=2, space="PSUM"))
    psX = ctx.enter_context(tc.tile_pool(name="psX", bufs=1, space="PSUM"))
    psO = ctx.enter_context(tc.tile_pool(name="psO", bufs=2, space="PSUM"))

    yacc_live = {}  # i -> psum tile for current chunk
    yg_live = {}    # i -> per-chunk gated-output tile [128, CH]
    dtx_live = {}   # i -> per-chunk dt*x tile [128, CH]

    def in_proj(c, i):
        """x/z projections for chunk c, d-tile i -> xpre[i], sz[i]."""
        csl = slice(c * CH, (c + 1) * CH)
        dsl = slice(i * 128, (i + 1) * 128)
        psx = psA.tile([128, CH], FP32, tag="inp", name=f"psx{c}_{i}")
        psz = psA.tile([128, CH], FP32, tag="inp", name=f"psz{c}_{i}")
        for ki in range(KT):
            hst = hs_pool.tile([128, CH], BF16, tag="hs")
            nc.sync.dma_start(hst[:], hsT[ki, :, csl])
            st = (ki == 0)
            sp = (ki == KT - 1)
            nc.tensor.matmul(psx[:], wx_sb[:, ki, dsl], hst[:], start=st, stop=sp)
            nc.tensor.matmul(psz[:], wz_sb[:, ki, dsl], hst[:], start=st, stop=sp)
        nc.scalar.copy(xpre[i][:, csl], psx[:])
        nc.scalar.activation(sz[i][:, csl], psz[:], ACTF.Silu)

    def conv(c, i):
        """causal depthwise conv over chunk c for d-tile i -> xact[i]."""
        bs = c * CH
        obs = bs % L  # offset within the batch
        acc = misc_pool.tile([128, CH], BF16, tag="cacc")
        nc.vector.tensor_scalar(acc[:], xpre[i][:, bs:bs + CH],
                                convw_sb[:, i, 3:4], convb_sb[:, i, :],
                                op0=ALU.mult, op1=ALU.add)
        for sh in (1, 2, 3):
            w = convw_sb[:, i, 3 - sh:4 - sh]
            if obs >= sh:
                nc.vector.scalar_tensor_tensor(
                    acc[:], xpre[i][:, bs - sh:bs + CH - sh], w, acc[:],
                    op0=ALU.mult, op1=ALU.add)
            else:
                nc.vector.scalar_tensor_tensor(
                    acc[:, sh:], xpre[i][:, bs:bs + CH - sh], w, acc[:, sh:],
                    op0=ALU.mult, op1=ALU.add)
        nc.scalar.activation(xact[i][:, bs:bs + CH], acc[:], ACTF.Silu)

    def x_proj_ar(c):
        """x_proj partials + chunked AllReduce for chunk c."""
        csl = slice(c * CH, (c + 1) * CH)
        ps0 = psX.tile([128, CH], FP32, tag="xpb")
        ps1 = psX.tile([2 * N, CH], FP32, tag="xps")
        for i in range(DT):
            nc.tensor.matmul(ps0[:], xp_sb[:, i, :R], xact[i][:, csl],
                             start=(i == 0), stop=(i == DT - 1))
            nc.tensor.matmul(ps1[:], xp_sb[:, i, R:], xact[i][:, csl],
                             start=(i == 0), stop=(i == DT - 1))
        st0 = misc_pool.tile([128, CH], BF16, tag="xst0")
        nc.scalar.copy(st0[:], ps0[:])
        st1 = misc_pool.tile([2 * N, CH], BF16, tag="xst1")
        nc.scalar.copy(st1[:], ps1[:])
        nc.sync.dma_start(xdb_part[c][:R, :], st0[:])
        nc.sync.dma_start(xdb_part[c][R:, :], st1[:])
        nc.gpsimd.collective_compute(
            "AllReduce", ALU.add,
            replica_groups=[list(range(cfg.n_cores))],
            ins=[xdb_part[c].opt()], outs=[xdb_red[c].opt()])

    def dt_proj(c):
        """dt_proj + softplus + dtx for chunk c."""
        csl = slice(c * CH, (c + 1) * CH)
        dtin = dtin_pool.tile([128, CH], BF16, tag="dtin")
        nc.sync.dma_start(dtin[:], xdb_red[c][:R, :])
        for i in range(DT):
            dsl = slice(i * 128, (i + 1) * 128)
            psd = psX.tile([128, CH], FP32, tag="xpb", name=f"psdt{c}_{i}")
            nc.tensor.matmul(psd[:], dtp_sb[:, dsl], dtin[:], start=True, stop=True)
            # softplus(x) = ln(1 + exp(x)); Exp and Ln share one act table
            et = misc_pool.tile([128, CH], FP32, tag="spexp")
            nc.scalar.activation(et[:], psd[:], ACTF.Exp, bias=dtb_sb[:, i, :])
            nc.scalar.activation(dt_sb[i][:, csl], et[:], ACTF.Ln, bias=1.0)
        for i in range(DT):
            dtxt = yg_pool.tile([128, CH], BF16, tag=f"dtx{i}", name=f"dtx{c}_{i}")
            dtx_live[i] = dtxt
            nc.vector.tensor_mul(dtxt[:], dt_sb[i][:, csl], xact[i][:, csl])

    def bcast(c):
        """broadcast B and C rows for chunk c across partitions."""
        bcb = bc_pool.tile([128, N, CH], BF16, tag="bcb", name=f"bcb{c}")
        bcc = bc_pool.tile([128, N, CH], BF16, tag="bcc", name=f"bcc{c}")
        for n in range(N):
            nc.sync.dma_start(bcb[:, n, :],
                              xdb_red[c][R + n:R + n + 1, :].to_broadcast((128, CH)))
            nc.sync.dma_start(bcc[:, n, :],
                              xdb_red[c][R + N + n:R + N + n + 1, :].to_broadcast((128, CH)))
        return bcb, bcc

    def scan_block(c, i, bcb, bcc):
        """16-state scan for chunk c, d-tile i; y accumulated into PSUM."""
        csl = slice(c * CH, (c + 1) * CH)
        yacc = psS.tile([128, CH], FP32, tag="yacc", name=f"yacc{c}_{i}")
        yacc_live[i] = yacc
        for n in range(N):
            dA = dA_pool.tile([128, CH], BF16, tag="dA")
            nc.scalar.activation(dA[:], dt_sb[i][:, csl], ACTF.Exp,
                                 scale=A_sb[:, i, n:n + 1])
            dBx = dbx_pool.tile([128, CH], BF16, tag="dBx")
            nc.vector.tensor_mul(dBx[:], dtx_live[i][:], bcb[:, n, :])
            h = h_pool.tile([128, CH], BF16, tag="h")
            hcol = i * N + n
            for s0 in range(0, CH, HF):
                seg = slice(s0, s0 + HF)
                if s0 == 0:
                    init = 0.0 if c % 2 == 0 else htail[:, hcol:hcol + 1]
                else:
                    init = h[:, s0 - 1:s0]
                nc.vector.tensor_tensor_scan(h[:, seg], dA[:, seg], dBx[:, seg],
                                             init, op0=ALU.mult, op1=ALU.add)
            if c % 2 == 0:
                nc.vector.tensor_copy(htail[:, hcol:hcol + 1], h[:, CH - 1:CH])
            hC = hc_pool.tile([128, CH], BF16, tag="hC")
            nc.vector.tensor_mul(hC[:], h[:], bcc[:, n, :])
            nc.tensor.matmul(yacc[:], id_sb[:], hC[:],
                             start=(n == 0), stop=(n == N - 1))

    def gating(c, i):
        """yg = (yacc + xact*D) * silu(z) for chunk c, d-tile i."""
        csl = slice(c * CH, (c + 1) * CH)
        tmp = misc_pool.tile([128, CH], BF16, tag="gtmp")
        nc.vector.scalar_tensor_tensor(tmp[:], xact[i][:, csl], Dv_sb[:, i, :],
                                       yacc_live[i][:], op0=ALU.mult, op1=ALU.add)
        ygt = yg_pool.tile([128, CH], BF16, tag=f"yg{i}", name=f"yg{c}_{i}")
        yg_live[i] = ygt
        nc.vector.tensor_mul(ygt[:], tmp[:], sz[i][:, csl])

    def out_proj(c):
        """out_proj for chunk c's tokens."""
        for tt in range(CH // 128):
            tok0 = c * CH + tt * 128
            tsl = slice(tt * 128, (tt + 1) * 128)
            for mc in range(DM // 512):
                msl = slice(mc * 512, (mc + 1) * 512)
                po = psO.tile([128, 512], FP32, tag="po")
                for i in range(DT):
                    nc.tensor.matmul(po[:], yg_live[i][:, tsl],
                                     wo_sb[:, i, msl],
                                     start=(i == 0), stop=(i == DT - 1))
                ost = misc_pool.tile([128, 512], FP32, tag="ost")
                nc.scalar.copy(ost[:], po[:])
                nc.sync.dma_start(outp[tok0:tok0 + 128, msl], ost[:])

    # ================= emission =================
    # prologue: chunk 0 front-end
    for i in range(DT):
        in_proj(0, i)
        conv(0, i)
    x_proj_ar(0)
    dt_proj(0)
    bc = bcast(0)

    for c in range(NCH):
        nxt = c + 1
        scan_block(c, 0, *bc)
        if nxt < NCH:
            in_proj(nxt, 0)
            conv(nxt, 0)
        gating(c, 0)
        scan_block(c, 1, *bc)
        if nxt < NCH:
            in_proj(nxt, 1)
            conv(nxt, 1)
        gating(c, 1)
        scan_block(c, 2, *bc)
        if nxt < NCH:
            in_proj(nxt, 2)
            conv(nxt, 2)
            in_proj(nxt, 3)
            conv(nxt, 3)
            x_proj_ar(nxt)
        gating(c, 2)
        scan_block(c, 3, *bc)
        if nxt < NCH:
            dt_proj(nxt)
            bc = bcast(nxt)
        gating(c, 3)
        out_proj(c)

    ctx.close()


# ===================== driver =====================
import numpy as np
import ml_dtypes

_N_CORES = 8
_B, _L, _DM = 2, 1024, 2048
_DI = 2 * _DM
_DC = _DI // _N_CORES
_N_STATE = 16
_R = _DM // 16

_compiled = None


def _get_compiled():
    global _compiled
    if _compiled is not None:
        return _compiled
    import concourse.bacc as bacc
    import concourse.tile as tile_mod
    cfg = Cfg(DM=_DM, DC=_DC, N=_N_STATE, R=_R, TOK=_B * _L, L=_L,
              n_cores=_N_CORES)
    nc = bacc.Bacc("TRN2", target_bir_lowering=False, debug=False,
                   num_devices=_N_CORES)
    io = declare_io(nc, cfg)
    with tile_mod.TileContext(nc) as tc:
        build(tc, io, cfg)
    nc.compile()
    _compiled = (nc, cfg)
    return _compiled


def _prep_in_maps(hidden_states, in_proj_w, conv_w, conv_b, x_proj_w,
                  dt_proj_w, dt_proj_b, A_log, D, out_proj_w):
    f32 = np.float32
    bf16 = ml_dtypes.bfloat16
    hs = np.ascontiguousarray(np.asarray(hidden_states, f32).reshape(_B * _L, _DM).T)
    in_proj_w = np.asarray(in_proj_w, f32)
    A = -np.exp(np.asarray(A_log, f32))
    x_proj_w = np.asarray(x_proj_w, f32)
    dt_proj_w = np.asarray(dt_proj_w, f32)
    out_proj_w = np.asarray(out_proj_w, f32)
    conv_w = np.asarray(conv_w, f32)
    conv_b = np.asarray(conv_b, f32)
    dt_proj_b = np.asarray(dt_proj_b, f32)
    D = np.asarray(D, f32)
    ident = np.eye(128, dtype=bf16)
    in_maps = []
    for c in range(_N_CORES):
        sl = slice(c * _DC, (c + 1) * _DC)
        in_maps.append({
            "hsT": hs.astype(bf16),
            "wxT": np.ascontiguousarray(in_proj_w[:_DI][sl].T).astype(bf16),
            "wzT": np.ascontiguousarray(in_proj_w[_DI:][sl].T).astype(bf16),
            "xpT": np.ascontiguousarray(x_proj_w[:, sl].T).astype(bf16),
            "dtpT": np.ascontiguousarray(dt_proj_w[sl].T).astype(bf16),
            "woT": np.ascontiguousarray(out_proj_w[:, sl].T).astype(bf16),
            "convw": np.ascontiguousarray(conv_w[sl]),
            "convb": np.ascontiguousarray(conv_b[sl][:, None]),
            "Amat": np.ascontiguousarray(A[sl]),
            "Dvec": np.ascontiguousarray(D[sl][:, None]),
            "dtb": np.ascontiguousarray(dt_proj_b[sl][:, None]),
            "ident": ident,
        })
    return in_maps


def kernel_run(trace=False, **inputs):
    from concourse import bass_utils
    nc, cfg = _get_compiled()
    in_maps = _prep_in_maps(**inputs)
    res = bass_utils.run_bass_kernel_spmd(
        nc, in_maps, core_ids=list(range(_N_CORES)), trace=trace)
    out = np.zeros((_B * _L, _DM), np.float64)
    for r in res.results:
        out += r["outp"].astype(np.float64)
    full = out.astype(np.float32).reshape(_B, _L, _DM)
    return full, res


def kernel(**inputs):
    full, _ = kernel_run(trace=False, **inputs)
    return full


# revision 24
# speedup vs baseline: 1.0945x; 1.0063x over previous
"""Trainium2 Bass kernel for nn_Jurassic3Mamba (Mamba-1 forward), 8-core SPMD.

v2: chunk-pipelined (512-token chunks), tensor-parallel over d_inner.
- All scan-phase elementwise ops in bf16 on DVE (no gpsimd -> no SBUF-port
  contention), dA on the scalar (ACT) engine.
- y = sum_n h_n*C_n accumulated in PSUM via identity-weight matmuls on the
  tensor engine (frees the vector engine of 15 adds per tile).
- AllReduce of x_dbl in bf16, one collective per 512-token chunk, launched
  ~75% through the previous chunk's scan so its latency is hidden.
- Native Silu / Softplus activations (one ACT op instead of sigmoid+mul /
  exp+ln pairs).
"""
import sys
if "/opt/trn_rl_repo" not in sys.path:
    sys.path.insert(0, "/opt/trn_rl_repo")


from contextlib import ExitStack

import concourse.bass as bass
import concourse.mybir as mybir
import concourse.tile as tile

FP32 = mybir.dt.float32
BF16 = mybir.dt.bfloat16
ALU = mybir.AluOpType
ACTF = mybir.ActivationFunctionType


class Cfg:
    def __init__(self, DM=2048, DC=512, N=16, R=128, TOK=2048, L=1024,
                 n_cores=8, scan_fd=256):
        self.DM = DM          # d_model
        self.DC = DC          # d_inner per core
        self.N = N            # d_state
        self.R = R            # dt_rank
        self.TOK = TOK        # B * L tokens
        self.L = L            # seq len per batch
        self.CH = 512         # chunk tokens
        self.n_cores = n_cores
        self.scan_fd = scan_fd
        assert DM % 128 == 0 and DC % 128 == 0 and R == 128
        self.KT = DM // 128   # k-tiles for in_proj contraction
        self.DT = DC // 128   # d-tiles per core
        self.NCH = TOK // self.CH  # chunks


def declare_io(nc, cfg):
    DM, DC, N, R, TOK = cfg.DM, cfg.DC, cfg.N, cfg.R, cfg.TOK
    io = {}
    io["hsT"] = nc.dram_tensor("hsT", [DM, TOK], BF16, kind="ExternalInput")
    io["wxT"] = nc.dram_tensor("wxT", [DM, DC], BF16, kind="ExternalInput")
    io["wzT"] = nc.dram_tensor("wzT", [DM, DC], BF16, kind="ExternalInput")
    io["xpT"] = nc.dram_tensor("xpT", [DC, R + 2 * N], BF16, kind="ExternalInput")
    io["dtpT"] = nc.dram_tensor("dtpT", [R, DC], BF16, kind="ExternalInput")
    io["woT"] = nc.dram_tensor("woT", [DC, DM], BF16, kind="ExternalInput")
    io["convw"] = nc.dram_tensor("convw", [DC, 4], FP32, kind="ExternalInput")
    io["convb"] = nc.dram_tensor("convb", [DC, 1], FP32, kind="ExternalInput")
    io["Amat"] = nc.dram_tensor("Amat", [DC, N], FP32, kind="ExternalInput")
    io["Dvec"] = nc.dram_tensor("Dvec", [DC, 1], FP32, kind="ExternalInput")
    io["dtb"] = nc.dram_tensor("dtb", [DC, 1], FP32, kind="ExternalInput")
    io["ident"] = nc.dram_tensor("ident", [128, 128], BF16, kind="ExternalInput")
    io["outp"] = nc.dram_tensor("outp", [TOK, DM], FP32, kind="ExternalOutput")
    return io


def build(tc: tile.TileContext, io, cfg: Cfg):
    nc = tc.nc
    ctx = ExitStack()
    DM, DC, N, R, TOK, L, CH = cfg.DM, cfg.DC, cfg.N, cfg.R, cfg.TOK, cfg.L, cfg.CH
    KT, DT, NCH = cfg.KT, cfg.DT, cfg.NCH
    HF = cfg.scan_fd  # scan segment length

    persist = ctx.enter_context(tc.tile_pool(name="persist", bufs=1))
    dram = ctx.enter_context(tc.tile_pool(name="dram", bufs=1, space="DRAM"))

    # ---- persistent weights ----
    xp_sb = persist.tile([128, DT, R + 2 * N], BF16, tag="xp")
    nc.sync.dma_start(xp_sb[:], io["xpT"].ap().rearrange("(t p) c -> p t c", p=128))
    dtp_sb = persist.tile([128, DC], BF16, tag="dtp")
    nc.sync.dma_start(dtp_sb[:], io["dtpT"].ap())
    wo_sb = persist.tile([128, DT, DM], BF16, tag="wo")
    nc.sync.dma_start(wo_sb[:], io["woT"].ap().rearrange("(t p) m -> p t m", p=128))
    wx_sb = persist.tile([128, KT, DC], BF16, tag="wx")
    nc.sync.dma_start(wx_sb[:], io["wxT"].ap().rearrange("(t p) c -> p t c", p=128))
    wz_sb = persist.tile([128, KT, DC], BF16, tag="wz")
    nc.sync.dma_start(wz_sb[:], io["wzT"].ap().rearrange("(t p) c -> p t c", p=128))
    convw_sb = persist.tile([128, DT, 4], FP32, tag="convw")
    nc.sync.dma_start(convw_sb[:], io["convw"].ap().rearrange("(t p) k -> p t k", p=128))
    convb_sb = persist.tile([128, DT, 1], FP32, tag="convb")
    nc.sync.dma_start(convb_sb[:], io["convb"].ap().rearrange("(t p) k -> p t k", p=128))
    A_sb = persist.tile([128, DT, N], FP32, tag="A")
    nc.sync.dma_start(A_sb[:], io["Amat"].ap().rearrange("(t p) n -> p t n", p=128))
    Dv_sb = persist.tile([128, DT, 1], FP32, tag="Dv")
    nc.sync.dma_start(Dv_sb[:], io["Dvec"].ap().rearrange("(t p) k -> p t k", p=128))
    dtb_sb = persist.tile([128, DT, 1], FP32, tag="dtb")
    nc.sync.dma_start(dtb_sb[:], io["dtb"].ap().rearrange("(t p) k -> p t k", p=128))
    id_sb = persist.tile([128, 128], BF16, tag="ident")
    nc.sync.dma_start(id_sb[:], io["ident"].ap())

    # persistent activations [128, TOK] bf16 per d-tile
    xpre = [persist.tile([128, TOK], BF16, tag=f"xpre{i}", name=f"xpre{i}") for i in range(DT)]
    xact = [persist.tile([128, TOK], BF16, tag=f"xact{i}", name=f"xact{i}") for i in range(DT)]
    sz = [persist.tile([128, TOK], BF16, tag=f"sz{i}", name=f"sz{i}") for i in range(DT)]
    dt_sb = [persist.tile([128, TOK], BF16, tag=f"dt{i}", name=f"dt{i}") for i in range(DT)]
    htail = persist.tile([128, DT * N], BF16, tag="htail")

    # per-chunk DRAM bounce buffers for the collective
    xdb_part = [dram.tile([R + 2 * N, CH], BF16, name=f"xdbp{c}") for c in range(NCH)]
    xdb_red = [dram.tile([R + 2 * N, CH], BF16, addr_space="Shared", name=f"xdbr{c}")
               for c in range(NCH)]

    hsT = io["hsT"].ap().rearrange("(t p) tok -> t p tok", p=128)  # [KT,128,TOK]
    outp = io["outp"].ap()

    # ---- working pools (whole-kernel scope) ----
    hs_pool = ctx.enter_context(tc.tile_pool(name="hs", bufs=6))
    bc_pool = ctx.enter_context(tc.tile_pool(name="bc", bufs=1))
    dtin_pool = ctx.enter_context(tc.tile_pool(name="dtin", bufs=2))
    dA_pool = ctx.enter_context(tc.tile_pool(name="dA", bufs=3))
    dbx_pool = ctx.enter_context(tc.tile_pool(name="dbx", bufs=2))
    h_pool = ctx.enter_context(tc.tile_pool(name="h", bufs=3))
    hc_pool = ctx.enter_context(tc.tile_pool(name="hc", bufs=12))
    yg_pool = ctx.enter_context(tc.tile_pool(name="ygp", bufs=2))
    misc_pool = ctx.enter_context(tc.tile_pool(name="misc", bufs=2))
    psA = ctx.enter_context(tc.tile_pool(name="psA", bufs=2, space="PSUM"))
    psS = ctx.enter_context(tc.tile_pool(name="psS", bufs=2, space="PSUM"))
    psX = ctx.enter_context(tc.tile_pool(name="psX", bufs=1, space="PSUM"))
    psO = ctx.enter_context(tc.tile_pool(name="psO", bufs=2, space="PSUM"))

    yacc_live = {}  # i -> psum tile for current chunk
    yg_live = {}    # i -> per-chunk gated-output tile [128, CH]
    dtx_live = {}   # i -> per-chunk dt*x tile [128, CH]

    def in_proj(c, i):
        """x/z projections for chunk c, d-tile i -> xpre[i], sz[i]."""
        csl = slice(c * CH, (c + 1) * CH)
        dsl = slice(i * 128, (i + 1) * 128)
        psx = psA.tile([128, CH], FP32, tag="inp", name=f"psx{c}_{i}")
        psz = psA.tile([128, CH], FP32, tag="inp", name=f"psz{c}_{i}")
        for ki in range(KT):
            hst = hs_pool.tile([128, CH], BF16, tag="hs")
            nc.sync.dma_start(hst[:], hsT[ki, :, csl])
            st = (ki == 0)
            sp = (ki == KT - 1)
            nc.tensor.matmul(psx[:], wx_sb[:, ki, dsl], hst[:], start=st, stop=sp)
            nc.tensor.matmul(psz[:], wz_sb[:, ki, dsl], hst[:], start=st, stop=sp)
        nc.scalar.copy(xpre[i][:, csl], psx[:])
        nc.scalar.copy(sz[i][:, csl], psz[:])  # raw z; Silu applied in silu_cluster

    def conv(c, i):
        """causal depthwise conv over chunk c for d-tile i -> xact[i]."""
        bs = c * CH
        obs = bs % L  # offset within the batch
        acc = xact[i][:, bs:bs + CH]  # raw conv result; Silu applied in silu_cluster
        nc.vector.tensor_scalar(acc, xpre[i][:, bs:bs + CH],
                                convw_sb[:, i, 3:4], convb_sb[:, i, :],
                                op0=ALU.mult, op1=ALU.add)
        for sh in (1, 2, 3):
            w = convw_sb[:, i, 3 - sh:4 - sh]
            if obs >= sh:
                nc.vector.scalar_tensor_tensor(
                    acc, xpre[i][:, bs - sh:bs + CH - sh], w, acc,
                    op0=ALU.mult, op1=ALU.add)
            else:
                nc.vector.scalar_tensor_tensor(
                    acc[:, sh:], xpre[i][:, bs:bs + CH - sh], w, acc[:, sh:],
                    op0=ALU.mult, op1=ALU.add)

    def silu_cluster(c):
        """Apply Silu in place to raw conv results and raw z for chunk c.

        Clustered so the Silu act-table is loaded once per chunk instead of
        being re-fetched between every scan block's Exp runs.
        """
        csl = slice(c * CH, (c + 1) * CH)
        for i in range(DT):
            nc.scalar.activation(xact[i][:, csl], xact[i][:, csl], ACTF.Silu)
        for i in range(DT):
            nc.scalar.activation(sz[i][:, csl], sz[i][:, csl], ACTF.Silu)

    def x_proj_ar(c):
        """x_proj partials + chunked AllReduce for chunk c."""
        csl = slice(c * CH, (c + 1) * CH)
        silu_cluster(c)
        ps0 = psX.tile([128, CH], FP32, tag="xpb")
        ps1 = psX.tile([2 * N, CH], FP32, tag="xps")
        for i in range(DT):
            nc.tensor.matmul(ps0[:], xp_sb[:, i, :R], xact[i][:, csl],
                             start=(i == 0), stop=(i == DT - 1))
            nc.tensor.matmul(ps1[:], xp_sb[:, i, R:], xact[i][:, csl],
                             start=(i == 0), stop=(i == DT - 1))
        st0 = misc_pool.tile([128, CH], BF16, tag="xst0")
        nc.scalar.copy(st0[:], ps0[:])
        st1 = misc_pool.tile([2 * N, CH], BF16, tag="xst1")
        nc.scalar.copy(st1[:], ps1[:])
        nc.sync.dma_start(xdb_part[c][:R, :], st0[:])
        nc.sync.dma_start(xdb_part[c][R:, :], st1[:])
        nc.gpsimd.collective_compute(
            "AllReduce", ALU.add,
            replica_groups=[list(range(cfg.n_cores))],
            ins=[xdb_part[c].opt()], outs=[xdb_red[c].opt()])

    def dt_proj(c):
        """dt_proj + softplus + dtx for chunk c."""
        csl = slice(c * CH, (c + 1) * CH)
        dtin = dtin_pool.tile([128, CH], BF16, tag="dtin")
        nc.sync.dma_start(dtin[:], xdb_red[c][:R, :])
        for i in range(DT):
            dsl = slice(i * 128, (i + 1) * 128)
            psd = psX.tile([128, CH], FP32, tag="xpb", name=f"psdt{c}_{i}")
            nc.tensor.matmul(psd[:], dtp_sb[:, dsl], dtin[:], start=True, stop=True)
            # softplus(x) = ln(1 + exp(x)); Exp and Ln share one act table
            et = misc_pool.tile([128, CH], FP32, tag="spexp")
            nc.scalar.activation(et[:], psd[:], ACTF.Exp, bias=dtb_sb[:, i, :])
            nc.scalar.activation(dt_sb[i][:, csl], et[:], ACTF.Ln, bias=1.0)
        for i in range(DT):
            dtxt = yg_pool.tile([128, CH], BF16, tag=f"dtx{i}", name=f"dtx{c}_{i}")
            dtx_live[i] = dtxt
            nc.vector.tensor_mul(dtxt[:], dt_sb[i][:, csl], xact[i][:, csl])

    def bcast(c):
        """broadcast B and C rows for chunk c across partitions."""
        bcb = bc_pool.tile([128, N, CH], BF16, tag="bcb", name=f"bcb{c}")
        bcc = bc_pool.tile([128, N, CH], BF16, tag="bcc", name=f"bcc{c}")
        for n in range(N):
            nc.sync.dma_start(bcb[:, n, :],
                              xdb_red[c][R + n:R + n + 1, :].to_broadcast((128, CH)))
            nc.sync.dma_start(bcc[:, n, :],
                              xdb_red[c][R + N + n:R + N + n + 1, :].to_broadcast((128, CH)))
        return bcb, bcc

    def scan_block(c, i, bcb, bcc):
        """16-state scan for chunk c, d-tile i; y accumulated into PSUM."""
        csl = slice(c * CH, (c + 1) * CH)
        yacc = psS.tile([128, CH], FP32, tag="yacc", name=f"yacc{c}_{i}")
        yacc_live[i] = yacc
        for n in range(N):
            dA = dA_pool.tile([128, CH], BF16, tag="dA")
            nc.scalar.activation(dA[:], dt_sb[i][:, csl], ACTF.Exp,
                                 scale=A_sb[:, i, n:n + 1])
            dBx = dbx_pool.tile([128, CH], BF16, tag="dBx")
            nc.vector.tensor_mul(dBx[:], dtx_live[i][:], bcb[:, n, :])
            h = h_pool.tile([128, CH], BF16, tag="h")
            hcol = i * N + n
            for s0 in range(0, CH, HF):
                seg = slice(s0, s0 + HF)
                if s0 == 0:
                    init = 0.0 if c % 2 == 0 else htail[:, hcol:hcol + 1]
                else:
                    init = h[:, s0 - 1:s0]
                nc.vector.tensor_tensor_scan(h[:, seg], dA[:, seg], dBx[:, seg],
                                             init, op0=ALU.mult, op1=ALU.add)
            if c % 2 == 0:
                nc.vector.tensor_copy(htail[:, hcol:hcol + 1], h[:, CH - 1:CH])
            hC = hc_pool.tile([128, CH], BF16, tag="hC")
            nc.vector.tensor_mul(hC[:], h[:], bcc[:, n, :])
            nc.tensor.matmul(yacc[:], id_sb[:], hC[:],
                             start=(n == 0), stop=(n == N - 1))

    def gating(c, i):
        """yg = (yacc + xact*D) * silu(z) for chunk c, d-tile i."""
        csl = slice(c * CH, (c + 1) * CH)
        tmp = misc_pool.tile([128, CH], BF16, tag="gtmp")
        nc.vector.scalar_tensor_tensor(tmp[:], xact[i][:, csl], Dv_sb[:, i, :],
                                       yacc_live[i][:], op0=ALU.mult, op1=ALU.add)
        ygt = yg_pool.tile([128, CH], BF16, tag=f"yg{i}", name=f"yg{c}_{i}")
        yg_live[i] = ygt
        nc.vector.tensor_mul(ygt[:], tmp[:], sz[i][:, csl])

    def out_proj(c):
        """out_proj for chunk c's tokens."""
        for tt in range(CH // 128):
            tok0 = c * CH + tt * 128
            tsl = slice(tt * 128, (tt + 1) * 128)
            for mc in range(DM // 512):
                msl = slice(mc * 512, (mc + 1) * 512)
                po = psO.tile([128, 512], FP32, tag="po")
                for i in range(DT):
                    nc.tensor.matmul(po[:], yg_live[i][:, tsl],
                                     wo_sb[:, i, msl],
                                     start=(i == 0), stop=(i == DT - 1))
                ost = misc_pool.tile([128, 512], FP32, tag="ost")
                nc.scalar.copy(ost[:], po[:])
                nc.sync.dma_start(outp[tok0:tok0 + 128, msl], ost[:])

    # ================= emission =================
    # prologue: chunk 0 front-end
    for i in range(DT):
        in_proj(0, i)
        conv(0, i)
    x_proj_ar(0)
    dt_proj(0)
    bc = bcast(0)

    pending_out = None
    for c in range(NCH):
        nxt = c + 1
        scan_block(c, 0, *bc)
        if pending_out is not None:
            out_proj(pending_out)
            pending_out = None
        if nxt < NCH:
            in_proj(nxt, 0)
            conv(nxt, 0)
        gating(c, 0)
        scan_block(c, 1, *bc)
        if nxt < NCH:
            in_proj(nxt, 1)
            conv(nxt, 1)
        gating(c, 1)
        scan_block(c, 2, *bc)
        if nxt < NCH:
            in_proj(nxt, 2)
            conv(nxt, 2)
            in_proj(nxt, 3)
            conv(nxt, 3)
            x_proj_ar(nxt)
        gating(c, 2)
        scan_block(c, 3, *bc)
        if nxt < NCH:
            dt_proj(nxt)
            bc = bcast(nxt)
        gating(c, 3)
        pending_out = c
    out_proj(NCH - 1)

    ctx.close()


# ===================== driver =====================
import numpy as np
import ml_dtypes

_N_CORES = 8
_B, _L, _DM = 2, 1024, 2048
_DI = 2 * _DM
_DC = _DI // _N_CORES
_N_STATE = 16
_R = _DM // 16

_compiled = None


def _get_compiled():
    global _compiled
    if _compiled is not None:
        return _compiled
    import concourse.bacc as bacc
    import concourse.tile as tile_mod
    cfg = Cfg(DM=_DM, DC=_DC, N=_N_STATE, R=_R, TOK=_B * _L, L=_L,
              n_cores=_N_CORES)
    nc = bacc.Bacc("TRN2", target_bir_lowering=False, debug=False,
                   num_devices=_N_CORES)
    io = declare_io(nc, cfg)
    with tile_mod.TileContext(nc) as tc:
        build(tc, io, cfg)
    nc.compile()
    _compiled = (nc, cfg)
    return _compiled


def _prep_in_maps(hidden_states, in_proj_w, conv_w, conv_b, x_proj_w,
                  dt_proj_w, dt_proj_b, A_log, D, out_proj_w):
    f32 = np.float32
    bf16 = ml_dtypes.bfloat16
    hs = np.ascontiguousarray(np.asarray(hidden_states, f32).reshape(_B * _L, _DM).T)
    in_proj_w = np.asarray(in_proj_w, f32)
    A = -np.exp(np.asarray(A_log, f32))
    x_proj_w = np.asarray(x_proj_w, f32)
    dt_proj_w = np.asarray(dt_proj_w, f32)
    out_proj_w = np.asarray(out_proj_w, f32)
    conv_w = np.asarray(conv_w, f32)
    conv_b = np.asarray(conv_b, f32)
    dt_proj_b = np.asarray(dt_proj_b, f32)
    D = np.asarray(D, f32)
    ident = np.eye(128, dtype=bf16)
    in_maps = []
    for c in range(_N_CORES):
        sl = slice(c * _DC, (c + 1) * _DC)
        in_maps.append({
            "hsT": hs.astype(bf16),
            "wxT": np.ascontiguousarray(in_proj_w[:_DI][sl].T).astype(bf16),
            "wzT": np.ascontiguousarray(in_proj_w[_DI:][sl].T).astype(bf16),
            "xpT": np.ascontiguousarray(x_proj_w[:, sl].T).astype(bf16),
            "dtpT": np.ascontiguousarray(dt_proj_w[sl].T).astype(bf16),
            "woT": np.ascontiguousarray(out_proj_w[:, sl].T).astype(bf16),
            "convw": np.ascontiguousarray(conv_w[sl]),
            "convb": np.ascontiguousarray(conv_b[sl][:, None]),
            "Amat": np.ascontiguousarray(A[sl]),
            "Dvec": np.ascontiguousarray(D[sl][:, None]),
            "dtb": np.ascontiguousarray(dt_proj_b[sl][:, None]),
            "ident": ident,
        })
    return in_maps


def kernel_run(trace=False, **inputs):
    from concourse import bass_utils
    nc, cfg = _get_compiled()
    in_maps = _prep_in_maps(**inputs)
    res = bass_utils.run_bass_kernel_spmd(
        nc, in_maps, core_ids=list(range(_N_CORES)), trace=trace)
    out = np.zeros((_B * _L, _DM), np.float64)
    for r in res.results:
        out += r["outp"].astype(np.float64)
    full = out.astype(np.float32).reshape(_B, _L, _DM)
    return full, res


def kernel(**inputs):
    full, _ = kernel_run(trace=False, **inputs)
    return full
